# revision 1
# baseline (speedup 1.0000x reference)
"""Trainium2 Bass kernel for nn_CONTRASTLoss: squared Pearson-correlation loss
over two 16,777,216-element f32 vectors.

Strategy (data-parallel over 8 NeuronCores):
  - Each core takes a contiguous 2,097,152-element shard of d1 and d2,
    viewed as [128 partitions x 16384 free], streamed in chunks (1 MiB DMAs,
    tapered smaller at the end to shorten the pipeline tail).
  - Per chunk, fused streaming ops compute per-partition partial sums of the
    five sufficient statistics around the known center 0.5 (avoids f32
    catastrophic cancellation in the covariance):
      VectorE : e1 = d1-0.5 (accum -> S1), e2 = d2-0.5 (accum -> S2),
                p = (d1-0.5)*e2 (accum -> P)
      ScalarE : Square(d1-0.5) via the activation bias port (accum -> Q1/Q2),
    so the two compute engines run fully decoupled; both are hidden under
    the HBM-bound DMA stream (~47us/core at ~358 GB/s).
  - Hand-scheduled raw Bass (no Tile framework): static 4-deep double
    buffering with per-buffer-slot DMA semaphores and engine flow-control
    sems. The tail is minimal: one merged [128 x 5*NCH] partials store and
    a GpSimd store-completion wait (the runtime zeroes semaphores at each
    execution start, so no in-kernel reset is needed).
  - Partials (~28 KB/core) are combined on host in float64.
"""
import sys

if '/opt/trn_rl_repo' not in sys.path:
    sys.path.insert(0, '/opt/trn_rl_repo')

import numpy as np

N = 16777216
NCORES = 8
SHARD = N // NCORES      # 2097152
P = 128
FTOT = SHARD // P        # 16384
CHUNKS = [2048] * 6 + [1024, 1024, 1024, 512, 512]
assert sum(CHUNKS) == FTOT
NCH = len(CHUNKS)
MAXF = max(CHUNKS)
TBUFS = 4

_cached_nc = None


def _build():
    import concourse.bacc as bacc
    import concourse.mybir as mybir

    f32 = mybir.dt.float32
    nc = bacc.Bacc('TRN2', target_bir_lowering=False, debug=False)

    cap = nc.alloc_sbuf_tensor("const_neghalf", [P, 1], f32)
    t1b = [nc.alloc_sbuf_tensor(f"t1b{i}", [P, MAXF], f32) for i in range(TBUFS)]
    t2b = [nc.alloc_sbuf_tensor(f"t2b{i}", [P, MAXF], f32) for i in range(TBUFS)]
    e1b = nc.alloc_sbuf_tensor("e1b", [P, MAXF], f32)
    e2b = nc.alloc_sbuf_tensor("e2b", [P, MAXF], f32)
    gpb = nc.alloc_sbuf_tensor("gpb", [P, MAXF], f32)
    sqb = nc.alloc_sbuf_tensor("sqb", [P, MAXF], f32)
    stats_t = nc.alloc_sbuf_tensor("stats_t", [P, 5 * NCH], f32)
    nc.const_aps.aps[(f32, -0.5)] = cap.ap()

    d1 = nc.declare_dram_parameter("d1", [P, FTOT], f32, isOutput=False)
    d2 = nc.declare_dram_parameter("d2", [P, FTOT], f32, isOutput=False)
    out = nc.declare_dram_parameter("stats", [5 * P * NCH], f32, isOutput=True)

    s1sem = [nc.alloc_semaphore(f"s1sem{i}") for i in range(TBUFS)]
    s2sem = [nc.alloc_semaphore(f"s2sem{i}") for i in range(TBUFS)]
    v_sem = nc.alloc_semaphore("v_sem")
    a_sem = nc.alloc_semaphore("a_sem")
    c_sem = nc.alloc_semaphore("c_sem")
    st_sem = nc.alloc_semaphore("st_sem")

    stv = stats_t.ap()
    s1 = stv[:, 0 * NCH:1 * NCH]
    s2 = stv[:, 1 * NCH:2 * NCH]
    pp = stv[:, 2 * NCH:3 * NCH]
    q1 = stv[:, 3 * NCH:4 * NCH]
    q2 = stv[:, 4 * NCH:5 * NCH]

    # --- init: the -0.5 bias constant for ScalarE Square ---
    nc.gpsimd.memset(cap.ap(), -0.5).then_inc(c_sem, 1)

    offs = np.cumsum([0] + CHUNKS)

    # --- SP: all load DMAs, flow-controlled on buffer-slot consumers ---
    for c, fch in enumerate(CHUNKS):
        if c >= TBUFS:
            nc.sync.wait_ge(v_sem, 3 * (c - TBUFS + 1))
            nc.sync.wait_ge(a_sem, 2 * (c - TBUFS + 1))
        lo = int(offs[c])
        t1 = t1b[c % TBUFS].ap()[:, :fch]
        t2 = t2b[c % TBUFS].ap()[:, :fch]
        nc.sync.dma_start(out=t1, in_=d1[:, lo:lo + fch]).then_inc(s1sem[c % TBUFS], 16)
        nc.sync.dma_start(out=t2, in_=d2[:, lo:lo + fch]).then_inc(s2sem[c % TBUFS], 16)

    # --- VectorE: ts(e1)+accum S1, ts(e2)+accum S2, stt +accum P ---
    for c, fch in enumerate(CHUNKS):
        t1 = t1b[c % TBUFS].ap()[:, :fch]
        t2 = t2b[c % TBUFS].ap()[:, :fch]
        e1 = e1b.ap()[:, :fch]
        e2 = e2b.ap()[:, :fch]
        gp = gpb.ap()[:, :fch]
        nc.vector.wait_ge(s1sem[c % TBUFS], 16 * (c // TBUFS + 1))
        nc.vector.tensor_scalar(
            out=e1, in0=t1, scalar1=0.5, scalar2=None,
            op0=mybir.AluOpType.subtract, op1=mybir.AluOpType.add,
            accum_out=s1[:, c:c + 1]).then_inc(v_sem, 1)
        nc.vector.wait_ge(s2sem[c % TBUFS], 16 * (c // TBUFS + 1))
        nc.vector.tensor_scalar(
            out=e2, in0=t2, scalar1=0.5, scalar2=None,
            op0=mybir.AluOpType.subtract, op1=mybir.AluOpType.add,
            accum_out=s2[:, c:c + 1]).then_inc(v_sem, 1)
        nc.vector.scalar_tensor_tensor(
            out=gp, in0=t1, scalar=0.5, in1=e2,
            op0=mybir.AluOpType.subtract, op1=mybir.AluOpType.mult,
            accum_out=pp[:, c:c + 1]).then_inc(v_sem, 1)

    # --- ScalarE: Square(t - 0.5) with accum -> Q1/Q2 ---
    nc.scalar.wait_ge(c_sem, 1)
    for c, fch in enumerate(CHUNKS):
        t1 = t1b[c % TBUFS].ap()[:, :fch]
        t2 = t2b[c % TBUFS].ap()[:, :fch]
        sq = sqb.ap()[:, :fch]
        nc.scalar.wait_ge(s1sem[c % TBUFS], 16 * (c // TBUFS + 1))
        nc.scalar.activation(
            out=sq, in_=t1, func=mybir.ActivationFunctionType.Square,
            bias=-0.5, scale=1.0,
            accum_out=q1[:, c:c + 1]).then_inc(a_sem, 1)
        nc.scalar.wait_ge(s2sem[c % TBUFS], 16 * (c // TBUFS + 1))
        nc.scalar.activation(
            out=sq, in_=t2, func=mybir.ActivationFunctionType.Square,
            bias=-0.5, scale=1.0,
            accum_out=q2[:, c:c + 1]).then_inc(a_sem, 1)

    # --- SP: single merged store once both producer engines finish ---
    ov = out[0:P * 5 * NCH].rearrange("(p c) -> p c", p=P)
    nc.sync.wait_ge(v_sem, 3 * NCH)
    nc.sync.wait_ge(a_sem, 2 * NCH)
    nc.sync.dma_start(out=ov, in_=stv).then_inc(st_sem, 16)

    # GpSimd proves store completion (keeps the kernel alive until the
    # output has landed in DRAM). No in-kernel semaphore reset: the runtime
    # zeroes semaphores at each execution start (verified by alternating
    # different-input executions of a no-clear build on shared devices).
    nc.gpsimd.wait_ge(st_sem, 16)

    nc.finalize()
    return nc


def _run_device(a1, a2, trace=False, tmpdir=None):
    from concourse.bass_utils import run_bass_kernel_spmd

    sh1 = a1.reshape(NCORES, P, FTOT)
    sh2 = a2.reshape(NCORES, P, FTOT)
    in_maps = [{"d1": sh1[c], "d2": sh2[c]} for c in range(NCORES)]
    global _cached_nc
    if _cached_nc is None:
        _cached_nc = _build()
    res = run_bass_kernel_spmd(
        _cached_nc, in_maps, list(range(NCORES)), trace=trace, tmpdir=tmpdir)
    stats = np.stack([res.results[c]["stats"] for c in range(NCORES)])
    return stats, res


def _combine(stats):
    # stats: [NCORES, 5*P*NCH] f32 partials around center 0.5,
    # per-core layout [P, 5, NCH] with stat order [S1, S2, P, Q1, Q2]
    t = stats.astype(np.float64).reshape(NCORES, P, 5, NCH)
    S1 = t[:, :, 0, :].sum()
    S2 = t[:, :, 1, :].sum()
    Pc = t[:, :, 2, :].sum()
    Q1 = t[:, :, 3, :].sum()
    Q2 = t[:, :, 4, :].sum()
    n = float(N)
    mean1c = S1 / n
    mean2c = S2 / n
    a1 = mean1c + 0.001
    a2 = mean2c + 0.001
    var1 = (Q1 - S1 * S1 / n) / (n - 1)
    var2 = (Q2 - S2 * S2 / n) / (n - 1)
    std1 = np.sqrt(var1)
    std2 = np.sqrt(var2)
    cov = (Pc - a2 * S1 - a1 * S2 + n * a1 * a2) / (n - 1)
    cor = cov / (std1 * std2 + 0.001)
    loss = 0.5 * (cor + 0.001) ** 2
    return np.array([loss], dtype=np.float32)


def kernel(distribution1, distribution2):
    a1 = np.ascontiguousarray(np.asarray(distribution1, dtype=np.float32))
    a2 = np.ascontiguousarray(np.asarray(distribution2, dtype=np.float32))
    stats, _ = _run_device(a1, a2)
    return _combine(stats)



# revision 2
# speedup vs baseline: 1.0497x; 1.0497x over previous
"""Trainium2 Bass kernel for nn_CONTRASTLoss v3: zoned fp16/f32 5-engine plan.

CoreSim v1 cost-model facts:
  - DMA occupies its trigger engine for out_free_bytes x 0.3855 ns. Pool
    (SWDGE) DMAs may cast f32->fp16 in flight, halving the charge
    (0.771 ns/col vs 1.542). SP and Act load f32 via HWDGE.
  - DVE fp16 gets 2x/4x modes: tensor_tensor 0.52 ns/col, tensor_scalar
    0.26 ns/col (f32 tensor_scalar: 0.52; two-input f32: 1.0417).
  - PE fp16 matmuls run 1 cycle/row at any p-state (0.4166 ns/col warm);
    LdWeights is free in-model. fp32 data cannot feed fp32r matmuls without
    an explicit rounding pass (BIR verifier), so PE consumes only fp16.
  - Act activation: 0.8533 ns/col (+187 ns accum, +1283 one-time Square
    table load); Pool compute 0.8333 ns/col.

Plan: chunks are zoned 'B' (fp16, cast-loaded by Pool) or 'F' (f32, loaded
by SP/Act). PE computes zone-B sums (ones-weights matmuls -> [128,512] PSUM)
and zone-B squares (diag-trick: lhsT=rhs=window -> [128,128] PSUM, diagonal
extracted once at the end with an affine_select identity mask). DVE computes
all cross-products P (fp16 tt+ts in zone B, f32 stt in zone F) and part of
the f32-zone sums; Act does f32-zone squares + rest of f32 sums; fp16
precision only ever touches zone-B moment partials (~0.02% effect on cov,
tolerance is 2e-2). Host combines raw moments in float64.
"""
import sys

if '/opt/trn_rl_repo' not in sys.path:
    sys.path.insert(0, '/opt/trn_rl_repo')

import numpy as np

N = 16777216
NCORES = 8
P = 128
FTOT = N // NCORES // P          # 16384
CH = [512, 1024, 2048, 2048, 2048, 2048, 2048, 2048, 1536, 512, 512]
assert sum(CH) == FTOT and all(c % 512 == 0 for c in CH)
NCH = len(CH)
CHOFF = np.cumsum([0] + CH).tolist()

KNOBS = dict(
    # zone per chunk: 'B' = fp16 via Pool cast-DMA, 'F' = f32 via SP/Act
    zone='BFBBFBFBBBF',
    # S_F units (t,c) assigned to Act (leading count, rest on DVE)
    sf_act_n=4,
    # Q_B units diverted from PE diag to Act squares (leading count)
    qb_act_n=1,
    # P_F chunks diverted from DVE to Pool stt (trailing count)
    pf_pool_n=0,
    # number of B chunks whose P goes to PE (diag on psP) instead of DVE
    pb_pe_n=0,
    # Q_B units diverted from PE diag to DVE tt+ts (trailing count)
    qb_dve_n=0,
    # last n B chunks: S and Q done on DVE (fp16) instead of PE, so the
    # PSUM banks close early and PE leaves the tail critical path
    tail_dve_nch=0,
    # number of F-zone loads given to Act instead of SP (trailing count)
    af_loads_n=0,
    lookahead=4.0,
    act_group=4096, dve_group=4096,
    af_first=True,    # Act loads d2 of the first F chunk (parallel fill)
    sred_act='split',  # 'act' | 'dve' | 'split': psS bank reduce placement
)

STATS_COLS = 48
_cached = None


def _plan(k):
    zone = k['zone']
    assert len(zone) == NCH and set(zone) <= {'B', 'F'}
    bch = [c for c in range(NCH) if zone[c] == 'B']
    fch = [c for c in range(NCH) if zone[c] == 'F']
    # zone-local column offsets
    zoff = {}
    ob = of = 0
    for c in range(NCH):
        if zone[c] == 'B':
            zoff[c] = ob
            ob += CH[c]
        else:
            zoff[c] = of
            of += CH[c]
    btot, ftot = ob, of

    deliv = [(t, c) for c in range(NCH) for t in (0, 1)]
    didx = {tc: i for i, tc in enumerate(deliv)}

    # loads: Pool gets every B (t,c); SP gets F (t,c) except trailing
    # af_loads_n which go to Act
    floads = [(t, c) for (t, c) in deliv if zone[c] == 'F']
    aset = set(floads[len(floads) - k['af_loads_n']:]) if k['af_loads_n'] \
        else set()
    if k.get('af_first') and fch:
        aset.add((1, fch[0]))
    qloads = {'S': [], 'A': [], 'P': []}
    avail = {}
    for (t, c) in deliv:
        q = 'P' if zone[c] == 'B' else ('A' if (t, c) in aset else 'S')
        qloads[q].append((t, c))
        avail[(t, c)] = (q, len(qloads[q]))
    return dict(zone=zone, bch=bch, fch=fch, zoff=zoff, btot=btot, ftot=ftot,
                deliv=deliv, didx=didx, qloads=qloads, avail=avail)


def _build(knobs=None):
    import concourse.bacc as bacc
    import concourse.mybir as mybir

    k = dict(KNOBS)
    if knobs:
        k.update(knobs)

    f32 = mybir.dt.float32
    f16 = mybir.dt.float16
    Alu = mybir.AluOpType
    Act = mybir.ActivationFunctionType
    nc = bacc.Bacc('TRN2', target_bir_lowering=False, debug=False)

    pl = _plan(k)
    zone, zoff = pl['zone'], pl['zoff']
    didx, avail, qloads = pl['didx'], pl['avail'], pl['qloads']

    d1 = nc.declare_dram_parameter("d1", [P, FTOT], f32, isOutput=False)
    d2 = nc.declare_dram_parameter("d2", [P, FTOT], f32, isOutput=False)
    out = nc.declare_dram_parameter("stats", [P * STATS_COLS], f32,
                                    isOutput=True)

    sbF = [nc.alloc_sbuf_tensor(f"sbF{t}", [P, max(pl['ftot'], 512)], f32)
           for t in (0, 1)]
    sbB = [nc.alloc_sbuf_tensor(f"sbB{t}", [P, max(pl['btot'], 512)], f16)
           for t in (0, 1)]
    scr_v = nc.alloc_sbuf_tensor("scr_v", [P, k['dve_group']], f32)
    scr_a = nc.alloc_sbuf_tensor("scr_a", [P, k['act_group']], f32)
    prod = nc.alloc_sbuf_tensor("prod", [P, 2048], f16)
    prod2 = nc.alloc_sbuf_tensor("prod2", [P, 2048], f16)
    stats = nc.alloc_sbuf_tensor("stats_sb", [P, STATS_COLS], f32)
    ones = nc.alloc_sbuf_tensor("ones_sb", [P, 128], f32)
    onesh = nc.alloc_sbuf_tensor("onesh_sb", [P, 128], f16)
    ident = nc.alloc_sbuf_tensor("ident_sb", [P, 128], f32)
    zero = nc.alloc_sbuf_tensor("zero_sb", [P, 1], f32)
    psS = [nc.alloc_psum_tensor(f"psS{t}", [P, 512], f32) for t in (0, 1)]
    psQ = [nc.alloc_psum_tensor(f"psQ{t}", [P, 128], f32) for t in (0, 1)]
    psP = nc.alloc_psum_tensor("psP", [P, 128], f32)
    nc.const_aps.aps[(f32, 0.0)] = zero.ap()

    qsem = {q: nc.alloc_semaphore(f"q{q}sem") for q in 'SAP'}
    c_sem = nc.alloc_semaphore("c_sem")
    pe_sem = nc.alloc_semaphore("pe_sem")
    v_sem = nc.alloc_semaphore("v_sem")
    a_sem = nc.alloc_semaphore("a_sem")
    p_sem = nc.alloc_semaphore("p_sem")
    st_sem = nc.alloc_semaphore("st_sem")

    drt = [d1, d2]

    colmap = {'P': [], 'Q1': [], 'Q2': [], 'S1': [], 'S2': [],
              'S1R': [], 'S2R': [], 'scratch': []}
    _next_col = [0]

    def col(stat):
        c = _next_col[0]
        _next_col[0] += 1
        assert c < STATS_COLS
        colmap[stat].append(c)
        return stats.ap()[:, c:c + 1]

    class Waits:
        def __init__(self, eng):
            self.eng = eng
            self.seen = {'S': 0, 'A': 0, 'P': 0}

        def need(self, reqs):
            for tc in reqs:
                q, kk = avail[tc]
                if kk > self.seen[q]:
                    self.eng.wait_ge(qsem[q], 16 * kk)
                    self.seen[q] = kk

    def sb(t, c):
        """(tensor, chunk) -> (sbuf AP slice, is_fp16)"""
        lo = zoff[c]
        hi = lo + CH[c]
        if zone[c] == 'B':
            return sbB[t].ap()[:, lo:hi], True
        return sbF[t].ap()[:, lo:hi], False

    def emit_sorted(engine_loads, compute_ops, emit_load):
        items = [(didx[tc] - k['lookahead'], ('L', tc)) for tc in engine_loads]
        items += [(kk + 0.25, ('C', fn)) for (kk, fn) in compute_ops]
        items.sort(key=lambda x: x[0])
        for _, (kind, payload) in items:
            if kind == 'L':
                emit_load(payload)
            else:
                payload()

    def mk_load(eng, q):
        def fn(tc):
            t, c = tc
            lo, hi = CHOFF[c], CHOFF[c + 1]
            ap, _ = sb(t, c)
            eng.dma_start(out=ap, in_=drt[t][:, lo:hi]).then_inc(qsem[q], 16)
        return fn

    # ---------------- consts on gpsimd (before its cast loads) --------------
    nc.gpsimd.memset(ones.ap(), 1.0).then_inc(c_sem, 1)
    nc.gpsimd.memset(zero.ap(), 0.0).then_inc(c_sem, 1)
    nc.gpsimd.memset(onesh.ap(), 1.0).then_inc(c_sem, 1)
    # identity: select ones[p,j] where j - p == 0 else 0
    nc.gpsimd.affine_select(
        out=ident.ap(), in_=ones.ap(), pattern=[[1, 128]],
        compare_op=Alu.is_equal, fill=0.0, base=0,
        channel_multiplier=-1).then_inc(c_sem, 1)

    # ---------------- SP: f32 loads ----------------
    for tc in qloads['S']:
        mk_load(nc.sync, 'S')(tc)

    # ---------------- work assignment lists ----------------
    # S_F units: (t, c) for F chunks; leading sf_act_n to Act, rest DVE
    sf_units = [(t, c) for c in pl['fch'] for t in (0, 1)]
    sf_units.sort(key=lambda tc: didx[tc])
    sf_act = set(sf_units[:k['sf_act_n']])
    # Q_B units: leading qb_act_n to Act squares, trailing qb_dve_n to DVE,
    # rest PE diag
    qb_units = [(t, c) for c in pl['bch'] for t in (0, 1)]
    qb_units.sort(key=lambda tc: didx[tc])
    qb_act = set(qb_units[:k['qb_act_n']])
    qb_dve = set(qb_units[len(qb_units) - k['qb_dve_n']:]) \
        if k['qb_dve_n'] else set()
    qb_dve -= qb_act
    # P_B chunks on PE: leading pb_pe_n of the B chunk list
    pb_pe = set(pl['bch'][:k['pb_pe_n']])
    # tail B chunks handled fully by DVE
    tail_b = set(pl['bch'][len(pl['bch']) - k['tail_dve_nch']:]) \
        if k['tail_dve_nch'] else set()
    qb_dve |= {(t, c) for c in tail_b for t in (0, 1)}
    qb_dve -= qb_act
    # P_F chunks: trailing pf_pool_n to Pool
    pf_pool = set(pl['fch'][len(pl['fch']) - k['pf_pool_n']:]) \
        if k['pf_pool_n'] else set()

    # ---------------- Act: loads + F squares + S_F copies + QB spill --------
    act_w = Waits(nc.scalar)
    n_act = 0
    act_ops = []

    def fruns(units, cap):
        """Group (t,c) units into runs of consecutive F chunks, same tensor,
        total width <= cap. Units must be F-zone."""
        out_runs = []
        cur = []
        curw = 0
        for (t, c) in units:
            ok = (cur and cur[-1][0] == t and curw + CH[c] <= cap and
                  pl['fch'].index(c) == pl['fch'].index(cur[-1][1]) + 1)
            if ok:
                cur.append((t, c))
                curw += CH[c]
            else:
                if cur:
                    out_runs.append(cur)
                cur = [(t, c)]
                curw = CH[c]
        if cur:
            out_runs.append(cur)
        return out_runs

    def fslice(t, run):
        lo = zoff[run[0][1]]
        hi = zoff[run[-1][1]] + CH[run[-1][1]]
        return sbF[t].ap()[:, lo:hi], hi - lo

    def mk_act_sq(run):
        def fn():
            t = run[0][0]
            ap, w = fslice(t, run)
            act_w.need(run)
            nc.scalar.wait_ge(c_sem, 2)
            nc.scalar.activation(
                out=scr_a.ap()[:, :w], in_=ap, func=Act.Square,
                bias=0.0, scale=1.0,
                accum_out=col('Q1' if t == 0 else 'Q2')).then_inc(a_sem, 1)
        return fn

    def mk_act_sqB(t, c):
        def fn():
            ap, _ = sb(t, c)
            act_w.need([(t, c)])
            nc.scalar.wait_ge(c_sem, 2)
            nc.scalar.activation(
                out=scr_a.ap()[:, :CH[c]], in_=ap, func=Act.Square,
                bias=0.0, scale=1.0,
                accum_out=col('Q1' if t == 0 else 'Q2')).then_inc(a_sem, 1)
        return fn

    def mk_act_scopy(run):
        def fn():
            t = run[0][0]
            ap, w = fslice(t, run)
            act_w.need(run)
            nc.scalar.activation(
                out=scr_a.ap()[:, :w], in_=ap, func=Act.Copy,
                accum_out=col('S1' if t == 0 else 'S2')).then_inc(a_sem, 1)
        return fn

    qf_units = [(t, c) for c in pl['fch'] for t in (0, 1)]
    qf_units.sort(key=lambda tc: didx[tc])
    for run in fruns(qf_units, k['act_group']):
        act_ops.append((didx[run[-1]], mk_act_sq(run)))
        n_act += 1
    for run in fruns(sorted(sf_act, key=lambda x: didx[x]), k['act_group']):
        act_ops.append((didx[run[-1]] + 0.1, mk_act_scopy(run)))
        n_act += 1
    for tc in sorted(qb_act, key=lambda x: didx[x]):
        act_ops.append((didx[tc] + 0.05, mk_act_sqB(*tc)))
        n_act += 1
    emit_sorted(qloads['A'], act_ops, mk_load(nc.scalar, 'A'))

    # ---------------- Pool: cast loads + P_F spill + keepalive --------------
    pool_w = Waits(nc.gpsimd)
    n_pool = 0
    pool_ops = []

    def mk_pool_p(c):
        def fn():
            a0, _ = sb(0, c)
            a1, _ = sb(1, c)
            pool_w.need([(0, c), (1, c)])
            nc.gpsimd.scalar_tensor_tensor(
                out=scr_v.ap()[:, :CH[c]], in0=a0, scalar=0.0, in1=a1,
                op0=Alu.subtract, op1=Alu.mult,
                accum_out=col('P')).then_inc(p_sem, 1)
        return fn

    for c in sorted(pf_pool):
        pool_ops.append((didx[(1, c)], mk_pool_p(c)))
        n_pool += 1
    emit_sorted(qloads['P'], pool_ops, mk_load(nc.gpsimd, 'P'))

    # ---------------- DVE: P everywhere + S_F rest + end reduces ------------
    dve_w = Waits(nc.vector)
    n_dve = 0
    dve_ops = []

    def mk_dve_pB(c):
        def fn():
            a0, _ = sb(0, c)
            a1, _ = sb(1, c)
            dve_w.need([(0, c), (1, c)])
            w = CH[c]
            nc.vector.tensor_tensor(
                out=prod.ap()[:, :w], in0=a0, in1=a1, op=Alu.mult)
            nc.vector.tensor_scalar(
                out=prod2.ap()[:, :w], in0=prod.ap()[:, :w],
                scalar1=1.0, scalar2=None, op0=Alu.mult, op1=Alu.add,
                accum_out=col('P')).then_inc(v_sem, 1)
        return fn

    def mk_dve_pF(c):
        def fn():
            a0, _ = sb(0, c)
            a1, _ = sb(1, c)
            dve_w.need([(0, c), (1, c)])
            nc.vector.scalar_tensor_tensor(
                out=scr_v.ap()[:, :CH[c]], in0=a0, scalar=0.0, in1=a1,
                op0=Alu.subtract, op1=Alu.mult,
                accum_out=col('P')).then_inc(v_sem, 1)
        return fn

    def mk_dve_sF(t, c):
        def fn():
            ap, _ = sb(t, c)
            dve_w.need([(t, c)])
            nc.vector.tensor_scalar(
                out=scr_v.ap()[:, :CH[c]], in0=ap, scalar1=0.0, scalar2=None,
                op0=Alu.add, op1=Alu.add,
                accum_out=col('S1' if t == 0 else 'S2')).then_inc(v_sem, 1)
        return fn

    def mk_dve_sB(t, c):
        def fn():
            ap, _ = sb(t, c)
            dve_w.need([(t, c)])
            w = CH[c]
            nc.vector.tensor_scalar(
                out=prod2.ap()[:, :w], in0=ap, scalar1=1.0, scalar2=None,
                op0=Alu.mult, op1=Alu.add,
                accum_out=col('S1' if t == 0 else 'S2')).then_inc(v_sem, 1)
        return fn

    def mk_dve_qB(t, c):
        def fn():
            ap, _ = sb(t, c)
            dve_w.need([(t, c)])
            w = CH[c]
            nc.vector.tensor_tensor(
                out=prod.ap()[:, :w], in0=ap, in1=ap, op=Alu.mult)
            nc.vector.tensor_scalar(
                out=prod2.ap()[:, :w], in0=prod.ap()[:, :w],
                scalar1=1.0, scalar2=None, op0=Alu.mult, op1=Alu.add,
                accum_out=col('Q1' if t == 0 else 'Q2')).then_inc(v_sem, 1)
        return fn

    for c in range(NCH):
        if c in pf_pool or c in pb_pe:
            continue
        mk = mk_dve_pB if zone[c] == 'B' else mk_dve_pF
        dve_ops.append((didx[(1, c)], mk(c)))
        n_dve += 1
    for (t, c) in sorted(qb_dve, key=lambda tc: didx[tc]):
        dve_ops.append((didx[(t, c)] + 0.15, mk_dve_qB(t, c)))
        n_dve += 1
    for c in sorted(tail_b):
        for t in (0, 1):
            dve_ops.append((didx[(t, c)] + 0.18, mk_dve_sB(t, c)))
            n_dve += 1
    for tc in sf_units[k['sf_act_n']:]:
        dve_ops.append((didx[tc] + 0.1, mk_dve_sF(*tc)))
        n_dve += 1

    dve_ops.sort(key=lambda x: x[0])
    for _, fn in dve_ops:
        fn()

    # ---------------- PE: zone-B sums + diag squares ----------------
    pe_w = Waits(nc.tensor)
    nc.tensor.wait_ge(c_sem, 3)
    oh = onesh.ap()
    pe_bch = [c for c in pl['bch'] if c not in tail_b]
    pe_units = sorted(((didx[(t, c)], t, c)
                       for c in pe_bch for t in (0, 1)))
    lastS = {t: max((didx[(t, c)], c) for c in pe_bch)[1] for t in (0, 1)}
    qb_pe = [tc for tc in qb_units if tc not in qb_act and tc not in qb_dve]
    lastQ = {}
    for (t, c) in qb_pe:
        if t not in lastQ or didx[(t, c)] > didx[(t, lastQ[t])]:
            lastQ[t] = c
    firstS = {0: True, 1: True}
    firstQ = {0: True, 1: True}
    firstP = [True]
    lastPc = max(pb_pe, key=lambda c: didx[(1, c)]) if pb_pe else None
    n_pe_done = 0
    for _, t, c in pe_units:
        ap, _ = sb(t, c)
        pe_w.need([(t, c)])
        lo = zoff[c]
        sb_full = sbB[t].ap()
        # S matmuls: ones weights, 512-wide windows
        for off in range(lo, lo + CH[c], 512):
            is_last = (c == lastS[t] and off + 512 == lo + CH[c])
            mm = nc.tensor.matmul(
                out=psS[t].ap(), lhsT=oh, rhs=sb_full[:, off:off + 512],
                start=firstS[t], stop=is_last)
            firstS[t] = False
            if is_last:
                mm.then_inc(pe_sem, 1)
                n_pe_done += 1
        # P diag matmuls (on d2 arrival): lhsT = d1 win, rhs = d2 win
        if t == 1 and c in pb_pe:
            pe_w.need([(0, c), (1, c)])
            a0 = sbB[0].ap()
            a1 = sbB[1].ap()
            for off in range(lo, lo + CH[c], 128):
                is_lastp = (c == lastPc and off + 128 == lo + CH[c])
                mm = nc.tensor.matmul(
                    out=psP.ap(), lhsT=a0[:, off:off + 128],
                    rhs=a1[:, off:off + 128], start=firstP[0], stop=is_lastp)
                firstP[0] = False
                if is_lastp:
                    mm.then_inc(pe_sem, 1)
                    n_pe_done += 1
        # Q diag matmuls: 128-wide windows, lhsT = rhs = window
        if (t, c) in qb_act or (t, c) in qb_dve:
            continue
        for off in range(lo, lo + CH[c], 128):
            win = sb_full[:, off:off + 128]
            is_last = (c == lastQ.get(t) and off + 128 == lo + CH[c])
            mm = nc.tensor.matmul(
                out=psQ[t].ap(), lhsT=win, rhs=win,
                start=firstQ[t], stop=is_last)
            firstQ[t] = False
            if is_last:
                mm.then_inc(pe_sem, 1)
                n_pe_done += 1

    # ---------------- end reduces ----------------
    red_on = {'act': ('A', 'A'), 'dve': ('V', 'V'), 'split': ('A', 'V')}[
        k['sred_act'] if isinstance(k['sred_act'], str) else
        ('act' if k['sred_act'] else 'dve')]
    waited = {'A': False, 'V': False}
    for (t, stat), eng in zip(((0, 'S1R'), (1, 'S2R')), red_on):
        if eng == 'A':
            if not waited['A']:
                nc.scalar.wait_ge(pe_sem, n_pe_done)
                waited['A'] = True
            nc.scalar.activation(
                out=scr_a.ap()[:, :512], in_=psS[t].ap(), func=Act.Copy,
                accum_out=col(stat)).then_inc(a_sem, 1)
            n_act += 1
        else:
            if not waited['V']:
                nc.vector.wait_ge(pe_sem, n_pe_done)
                waited['V'] = True
            nc.vector.tensor_scalar(
                out=scr_v.ap()[:, :512], in0=psS[t].ap(), scalar1=0.0,
                scalar2=None, op0=Alu.add, op1=Alu.add,
                accum_out=col(stat)).then_inc(v_sem, 1)
            n_dve += 1
    if pb_pe:
        nc.vector.wait_ge(pe_sem, n_pe_done)
        nc.vector.tensor_scalar(
            out=scr_v.ap()[:, :128], in0=psP.ap(), scalar1=1.0,
            scalar2=None, op0=Alu.mult, op1=Alu.add,
            accum_out=col('scratch'))
        nc.vector.scalar_tensor_tensor(
            out=scr_v.ap()[:, 128:256], in0=scr_v.ap()[:, :128],
            scalar=0.0, in1=ident.ap(), op0=Alu.subtract, op1=Alu.mult,
            accum_out=col('P')).then_inc(v_sem, 1)
        n_dve += 1
    if qb_pe:
        # ttr crashes HW; extract the diagonal via psum-copy + stt*identity
        for t, stat in ((0, 'Q1'), (1, 'Q2')):
            nc.vector.tensor_scalar(
                out=scr_v.ap()[:, :128], in0=psQ[t].ap(), scalar1=1.0,
                scalar2=None, op0=Alu.mult, op1=Alu.add,
                accum_out=col('scratch'))
            nc.vector.scalar_tensor_tensor(
                out=scr_v.ap()[:, 128:256], in0=scr_v.ap()[:, :128],
                scalar=0.0, in1=ident.ap(), op0=Alu.subtract, op1=Alu.mult,
                accum_out=col(stat)).then_inc(v_sem, 1)
            n_dve += 1

    # ---------------- SP: final store; Pool: keepalive ----------------
    nc.sync.wait_ge(v_sem, n_dve)
    if n_act:
        nc.sync.wait_ge(a_sem, n_act)
    if n_pool:
        nc.sync.wait_ge(p_sem, n_pool)
    ov = out[0:P * STATS_COLS].rearrange("(p c) -> p c", p=P)
    nc.sync.dma_start(out=ov, in_=stats.ap()).then_inc(st_sem, 16)

    nc.gpsimd.wait_ge(st_sem, 16)

    nc.finalize()
    return nc, colmap


def _run_device(a1, a2, trace=False, tmpdir=None):
    from concourse.bass_utils import run_bass_kernel_spmd

    sh1 = a1.reshape(NCORES, P, FTOT)
    sh2 = a2.reshape(NCORES, P, FTOT)
    in_maps = [{"d1": sh1[c], "d2": sh2[c]} for c in range(NCORES)]
    global _cached
    if _cached is None:
        _cached = _build()
    nc, colmap = _cached
    res = run_bass_kernel_spmd(
        nc, in_maps, list(range(NCORES)), trace=trace, tmpdir=tmpdir)
    raw = np.stack([res.results[c]["stats"] for c in range(NCORES)])
    return raw, colmap, res


def _combine(raw, colmap):
    t = raw.astype(np.float64).reshape(NCORES, P, STATS_COLS)
    Pc = t[:, :, colmap['P']].sum()
    Q1 = t[:, :, colmap['Q1']].sum()
    Q2 = t[:, :, colmap['Q2']].sum()
    S1 = t[:, :, colmap['S1']].sum() + t[:, 0, colmap['S1R']].sum()
    S2 = t[:, :, colmap['S2']].sum() + t[:, 0, colmap['S2R']].sum()
    n = float(N)
    a1 = S1 / n + 0.001
    a2 = S2 / n + 0.001
    var1 = (Q1 - S1 * S1 / n) / (n - 1)
    var2 = (Q2 - S2 * S2 / n) / (n - 1)
    std1 = np.sqrt(var1)
    std2 = np.sqrt(var2)
    cov = (Pc - a2 * S1 - a1 * S2 + n * a1 * a2) / (n - 1)
    cor = cov / (std1 * std2 + 0.001)
    loss = 0.5 * (cor + 0.001) ** 2
    return np.array([loss], dtype=np.float32)


def kernel(distribution1, distribution2):
    a1 = np.ascontiguousarray(np.asarray(distribution1, dtype=np.float32))
    a2 = np.ascontiguousarray(np.asarray(distribution2, dtype=np.float32))
    raw, colmap, _ = _run_device(a1, a2)
    return _combine(raw, colmap)


# revision 3
# speedup vs baseline: 1.0952x; 1.0433x over previous
"""Trainium2 Bass kernel for nn_CONTRASTLoss v3: zoned fp16/f32 5-engine plan.

CoreSim v1 cost-model facts:
  - DMA occupies its trigger engine for out_free_bytes x 0.3855 ns. Pool
    (SWDGE) DMAs may cast f32->fp16 in flight, halving the charge
    (0.771 ns/col vs 1.542). SP and Act load f32 via HWDGE.
  - DVE fp16 gets 2x/4x modes: tensor_tensor 0.52 ns/col, tensor_scalar
    0.26 ns/col (f32 tensor_scalar: 0.52; two-input f32: 1.0417).
  - PE fp16 matmuls run 1 cycle/row at any p-state (0.4166 ns/col warm);
    LdWeights is free in-model. fp32 data cannot feed fp32r matmuls without
    an explicit rounding pass (BIR verifier), so PE consumes only fp16.
  - Act activation: 0.8533 ns/col (+187 ns accum, +1283 one-time Square
    table load); Pool compute 0.8333 ns/col.

Plan: chunks are zoned 'B' (fp16, cast-loaded by Pool) or 'F' (f32, loaded
by SP/Act). PE computes zone-B sums (ones-weights matmuls -> [128,512] PSUM)
and zone-B squares (diag-trick: lhsT=rhs=window -> [128,128] PSUM, diagonal
extracted once at the end with an affine_select identity mask). DVE computes
all cross-products P (fp16 tt+ts in zone B, f32 stt in zone F) and part of
the f32-zone sums; Act does f32-zone squares + rest of f32 sums; fp16
precision only ever touches zone-B moment partials (~0.02% effect on cov,
tolerance is 2e-2). Host combines raw moments in float64.
"""
import sys

if '/opt/trn_rl_repo' not in sys.path:
    sys.path.insert(0, '/opt/trn_rl_repo')

import numpy as np

N = 16777216
NCORES = 8
P = 128
FTOT = N // NCORES // P          # 16384
CH = [512, 1024, 1536, 1536, 1536, 1536, 1536, 1536, 1536, 1536, 1024, 1024, 512]
assert sum(CH) == FTOT and all(c % 512 == 0 for c in CH)
NCH = len(CH)
CHOFF = np.cumsum([0] + CH).tolist()

KNOBS = dict(
    # zone per chunk: 'B' = fp16 via Pool cast-DMA, 'F' = f32 via SP/Act
    zone='BFBBFBBFBBFBB',
    # S_F units (t,c) assigned to Act (leading count, rest on DVE)
    sf_act_n=5,
    # Q_B units diverted from PE diag to Act squares (leading count)
    qb_act_n=0,
    # P_F chunks diverted from DVE to Pool stt (trailing count)
    pf_pool_n=0,
    # number of B chunks whose P goes to PE (diag on psP) instead of DVE
    pb_pe_n=0,
    # Q_B units diverted from PE diag to DVE tt+ts (trailing count)
    qb_dve_n=0,
    # last n B chunks: S and Q done on DVE (fp16) instead of PE, so the
    # PSUM banks close early and PE leaves the tail critical path
    tail_dve_nch=0,
    # number of F-zone loads given to Act instead of SP (trailing count)
    af_loads_n=0,
    lookahead=4.0,
    act_group=4096, dve_group=4096,
    af_first=True,    # Act loads d2 of the first F chunk (parallel fill)
    sred_act='split',  # 'act' | 'dve' | 'split': psS bank reduce placement
)

STATS_COLS = 48
_cached = None


def _plan(k):
    zone = k['zone']
    assert len(zone) == NCH and set(zone) <= {'B', 'F'}
    bch = [c for c in range(NCH) if zone[c] == 'B']
    fch = [c for c in range(NCH) if zone[c] == 'F']
    # zone-local column offsets
    zoff = {}
    ob = of = 0
    for c in range(NCH):
        if zone[c] == 'B':
            zoff[c] = ob
            ob += CH[c]
        else:
            zoff[c] = of
            of += CH[c]
    btot, ftot = ob, of

    deliv = [(t, c) for c in range(NCH) for t in (0, 1)]
    didx = {tc: i for i, tc in enumerate(deliv)}

    # loads: Pool gets every B (t,c); SP gets F (t,c) except trailing
    # af_loads_n which go to Act
    floads = [(t, c) for (t, c) in deliv if zone[c] == 'F']
    aset = set(floads[len(floads) - k['af_loads_n']:]) if k['af_loads_n'] \
        else set()
    if k.get('af_first') and fch:
        aset.add((1, fch[0]))
    qloads = {'S': [], 'A': [], 'P': []}
    avail = {}
    for (t, c) in deliv:
        q = 'P' if zone[c] == 'B' else ('A' if (t, c) in aset else 'S')
        qloads[q].append((t, c))
        avail[(t, c)] = (q, len(qloads[q]))
    return dict(zone=zone, bch=bch, fch=fch, zoff=zoff, btot=btot, ftot=ftot,
                deliv=deliv, didx=didx, qloads=qloads, avail=avail)


def _build(knobs=None):
    import concourse.bacc as bacc
    import concourse.mybir as mybir

    k = dict(KNOBS)
    if knobs:
        k.update(knobs)

    f32 = mybir.dt.float32
    f16 = mybir.dt.float16
    Alu = mybir.AluOpType
    Act = mybir.ActivationFunctionType
    nc = bacc.Bacc('TRN2', target_bir_lowering=False, debug=False)

    pl = _plan(k)
    zone, zoff = pl['zone'], pl['zoff']
    didx, avail, qloads = pl['didx'], pl['avail'], pl['qloads']

    d1 = nc.declare_dram_parameter("d1", [P, FTOT], f32, isOutput=False)
    d2 = nc.declare_dram_parameter("d2", [P, FTOT], f32, isOutput=False)
    out = nc.declare_dram_parameter("stats", [P * STATS_COLS], f32,
                                    isOutput=True)

    sbF = [nc.alloc_sbuf_tensor(f"sbF{t}", [P, max(pl['ftot'], 512)], f32)
           for t in (0, 1)]
    sbB = [nc.alloc_sbuf_tensor(f"sbB{t}", [P, max(pl['btot'], 512)], f16)
           for t in (0, 1)]
    scr_v = nc.alloc_sbuf_tensor("scr_v", [P, k['dve_group']], f32)
    scr_a = nc.alloc_sbuf_tensor("scr_a", [P, k['act_group']], f32)
    prod = nc.alloc_sbuf_tensor("prod", [P, 2048], f16)
    prod2 = nc.alloc_sbuf_tensor("prod2", [P, 2048], f16)
    stats = nc.alloc_sbuf_tensor("stats_sb", [P, STATS_COLS], f32)
    ones = nc.alloc_sbuf_tensor("ones_sb", [P, 128], f32)
    onesh = nc.alloc_sbuf_tensor("onesh_sb", [P, 128], f16)
    ident = nc.alloc_sbuf_tensor("ident_sb", [P, 128], f32)
    zero = nc.alloc_sbuf_tensor("zero_sb", [P, 1], f32)
    psS = [nc.alloc_psum_tensor(f"psS{t}", [P, 512], f32) for t in (0, 1)]
    psQ = [nc.alloc_psum_tensor(f"psQ{t}", [P, 128], f32) for t in (0, 1)]
    psP = nc.alloc_psum_tensor("psP", [P, 128], f32)
    nc.const_aps.aps[(f32, 0.0)] = zero.ap()

    qsem = {q: nc.alloc_semaphore(f"q{q}sem") for q in 'SAP'}
    c_sem = nc.alloc_semaphore("c_sem")
    pe_sem = nc.alloc_semaphore("pe_sem")
    v_sem = nc.alloc_semaphore("v_sem")
    a_sem = nc.alloc_semaphore("a_sem")
    p_sem = nc.alloc_semaphore("p_sem")
    st_sem = nc.alloc_semaphore("st_sem")

    drt = [d1, d2]

    colmap = {'P': [], 'Q1': [], 'Q2': [], 'S1': [], 'S2': [],
              'S1R': [], 'S2R': [], 'scratch': []}
    _next_col = [0]

    def col(stat):
        c = _next_col[0]
        _next_col[0] += 1
        assert c < STATS_COLS
        colmap[stat].append(c)
        return stats.ap()[:, c:c + 1]

    class Waits:
        def __init__(self, eng):
            self.eng = eng
            self.seen = {'S': 0, 'A': 0, 'P': 0}

        def need(self, reqs):
            for tc in reqs:
                q, kk = avail[tc]
                if kk > self.seen[q]:
                    self.eng.wait_ge(qsem[q], 16 * kk)
                    self.seen[q] = kk

    def sb(t, c):
        """(tensor, chunk) -> (sbuf AP slice, is_fp16)"""
        lo = zoff[c]
        hi = lo + CH[c]
        if zone[c] == 'B':
            return sbB[t].ap()[:, lo:hi], True
        return sbF[t].ap()[:, lo:hi], False

    def emit_sorted(engine_loads, compute_ops, emit_load):
        items = [(didx[tc] - k['lookahead'], ('L', tc)) for tc in engine_loads]
        items += [(kk + 0.25, ('C', fn)) for (kk, fn) in compute_ops]
        items.sort(key=lambda x: x[0])
        for _, (kind, payload) in items:
            if kind == 'L':
                emit_load(payload)
            else:
                payload()

    def mk_load(eng, q):
        def fn(tc):
            t, c = tc
            lo, hi = CHOFF[c], CHOFF[c + 1]
            ap, _ = sb(t, c)
            eng.dma_start(out=ap, in_=drt[t][:, lo:hi]).then_inc(qsem[q], 16)
        return fn

    # ---------------- consts on gpsimd (before its cast loads) --------------
    nc.gpsimd.memset(ones.ap(), 1.0).then_inc(c_sem, 1)
    nc.gpsimd.memset(zero.ap(), 0.0).then_inc(c_sem, 1)
    nc.gpsimd.memset(onesh.ap(), 1.0).then_inc(c_sem, 1)
    # identity: select ones[p,j] where j - p == 0 else 0
    nc.gpsimd.affine_select(
        out=ident.ap(), in_=ones.ap(), pattern=[[1, 128]],
        compare_op=Alu.is_equal, fill=0.0, base=0,
        channel_multiplier=-1).then_inc(c_sem, 1)

    # ---------------- SP: f32 loads ----------------
    for tc in qloads['S']:
        mk_load(nc.sync, 'S')(tc)

    # ---------------- work assignment lists ----------------
    # S_F units: (t, c) for F chunks; leading sf_act_n to Act, rest DVE
    sf_units = [(t, c) for c in pl['fch'] for t in (0, 1)]
    sf_units.sort(key=lambda tc: didx[tc])
    sf_act = set(sf_units[:k['sf_act_n']])
    # Q_B units: leading qb_act_n to Act squares, trailing qb_dve_n to DVE,
    # rest PE diag
    qb_units = [(t, c) for c in pl['bch'] for t in (0, 1)]
    qb_units.sort(key=lambda tc: didx[tc])
    qb_act = set(qb_units[:k['qb_act_n']])
    qb_dve = set(qb_units[len(qb_units) - k['qb_dve_n']:]) \
        if k['qb_dve_n'] else set()
    qb_dve -= qb_act
    # P_B chunks on PE: leading pb_pe_n of the B chunk list
    pb_pe = set(pl['bch'][:k['pb_pe_n']])
    # tail B chunks handled fully by DVE
    tail_b = set(pl['bch'][len(pl['bch']) - k['tail_dve_nch']:]) \
        if k['tail_dve_nch'] else set()
    qb_dve |= {(t, c) for c in tail_b for t in (0, 1)}
    qb_dve -= qb_act
    # P_F chunks: trailing pf_pool_n to Pool
    pf_pool = set(pl['fch'][len(pl['fch']) - k['pf_pool_n']:]) \
        if k['pf_pool_n'] else set()

    # ---------------- Act: loads + F squares + S_F copies + QB spill --------
    act_w = Waits(nc.scalar)
    n_act = 0
    act_ops = []

    def fruns(units, cap):
        """Group (t,c) units into runs of consecutive F chunks, same tensor,
        total width <= cap. Units must be F-zone."""
        out_runs = []
        cur = []
        curw = 0
        for (t, c) in units:
            ok = (cur and cur[-1][0] == t and curw + CH[c] <= cap and
                  pl['fch'].index(c) == pl['fch'].index(cur[-1][1]) + 1)
            if ok:
                cur.append((t, c))
                curw += CH[c]
            else:
                if cur:
                    out_runs.append(cur)
                cur = [(t, c)]
                curw = CH[c]
        if cur:
            out_runs.append(cur)
        return out_runs

    def fslice(t, run):
        lo = zoff[run[0][1]]
        hi = zoff[run[-1][1]] + CH[run[-1][1]]
        return sbF[t].ap()[:, lo:hi], hi - lo

    def mk_act_sq(run):
        def fn():
            t = run[0][0]
            ap, w = fslice(t, run)
            act_w.need(run)
            nc.scalar.wait_ge(c_sem, 2)
            nc.scalar.activation(
                out=scr_a.ap()[:, :w], in_=ap, func=Act.Square,
                bias=0.0, scale=1.0,
                accum_out=col('Q1' if t == 0 else 'Q2')).then_inc(a_sem, 1)
        return fn

    def mk_act_sqB(t, c):
        def fn():
            ap, _ = sb(t, c)
            act_w.need([(t, c)])
            nc.scalar.wait_ge(c_sem, 2)
            nc.scalar.activation(
                out=scr_a.ap()[:, :CH[c]], in_=ap, func=Act.Square,
                bias=0.0, scale=1.0,
                accum_out=col('Q1' if t == 0 else 'Q2')).then_inc(a_sem, 1)
        return fn

    def mk_act_scopy(run):
        def fn():
            t = run[0][0]
            ap, w = fslice(t, run)
            act_w.need(run)
            nc.scalar.activation(
                out=scr_a.ap()[:, :w], in_=ap, func=Act.Copy,
                accum_out=col('S1' if t == 0 else 'S2')).then_inc(a_sem, 1)
        return fn

    qf_units = [(t, c) for c in pl['fch'] for t in (0, 1)]
    qf_units.sort(key=lambda tc: didx[tc])
    for run in fruns(qf_units, k['act_group']):
        act_ops.append((didx[run[-1]], mk_act_sq(run)))
        n_act += 1
    for run in fruns(sorted(sf_act, key=lambda x: didx[x]), k['act_group']):
        act_ops.append((didx[run[-1]] + 0.1, mk_act_scopy(run)))
        n_act += 1
    for tc in sorted(qb_act, key=lambda x: didx[x]):
        act_ops.append((didx[tc] + 0.05, mk_act_sqB(*tc)))
        n_act += 1
    emit_sorted(qloads['A'], act_ops, mk_load(nc.scalar, 'A'))

    # ---------------- Pool: cast loads + P_F spill + keepalive --------------
    pool_w = Waits(nc.gpsimd)
    n_pool = 0
    pool_ops = []

    def mk_pool_p(c):
        def fn():
            a0, _ = sb(0, c)
            a1, _ = sb(1, c)
            pool_w.need([(0, c), (1, c)])
            nc.gpsimd.scalar_tensor_tensor(
                out=scr_v.ap()[:, :CH[c]], in0=a0, scalar=0.0, in1=a1,
                op0=Alu.subtract, op1=Alu.mult,
                accum_out=col('P')).then_inc(p_sem, 1)
        return fn

    for c in sorted(pf_pool):
        pool_ops.append((didx[(1, c)], mk_pool_p(c)))
        n_pool += 1
    emit_sorted(qloads['P'], pool_ops, mk_load(nc.gpsimd, 'P'))

    # ---------------- DVE: P everywhere + S_F rest + end reduces ------------
    dve_w = Waits(nc.vector)
    n_dve = 0
    dve_ops = []

    def mk_dve_pB(c):
        def fn():
            a0, _ = sb(0, c)
            a1, _ = sb(1, c)
            dve_w.need([(0, c), (1, c)])
            w = CH[c]
            nc.vector.tensor_tensor(
                out=prod.ap()[:, :w], in0=a0, in1=a1, op=Alu.mult)
            nc.vector.tensor_scalar(
                out=prod2.ap()[:, :w], in0=prod.ap()[:, :w],
                scalar1=1.0, scalar2=None, op0=Alu.mult, op1=Alu.add,
                accum_out=col('P')).then_inc(v_sem, 1)
        return fn

    def mk_dve_pF(c):
        def fn():
            a0, _ = sb(0, c)
            a1, _ = sb(1, c)
            dve_w.need([(0, c), (1, c)])
            nc.vector.scalar_tensor_tensor(
                out=scr_v.ap()[:, :CH[c]], in0=a0, scalar=0.0, in1=a1,
                op0=Alu.subtract, op1=Alu.mult,
                accum_out=col('P')).then_inc(v_sem, 1)
        return fn

    def mk_dve_sF(t, c):
        def fn():
            ap, _ = sb(t, c)
            dve_w.need([(t, c)])
            nc.vector.tensor_scalar(
                out=scr_v.ap()[:, :CH[c]], in0=ap, scalar1=0.0, scalar2=None,
                op0=Alu.add, op1=Alu.add,
                accum_out=col('S1' if t == 0 else 'S2')).then_inc(v_sem, 1)
        return fn

    def mk_dve_sB(t, c):
        def fn():
            ap, _ = sb(t, c)
            dve_w.need([(t, c)])
            w = CH[c]
            nc.vector.tensor_scalar(
                out=prod2.ap()[:, :w], in0=ap, scalar1=1.0, scalar2=None,
                op0=Alu.mult, op1=Alu.add,
                accum_out=col('S1' if t == 0 else 'S2')).then_inc(v_sem, 1)
        return fn

    def mk_dve_qB(t, c):
        def fn():
            ap, _ = sb(t, c)
            dve_w.need([(t, c)])
            w = CH[c]
            nc.vector.tensor_tensor(
                out=prod.ap()[:, :w], in0=ap, in1=ap, op=Alu.mult)
            nc.vector.tensor_scalar(
                out=prod2.ap()[:, :w], in0=prod.ap()[:, :w],
                scalar1=1.0, scalar2=None, op0=Alu.mult, op1=Alu.add,
                accum_out=col('Q1' if t == 0 else 'Q2')).then_inc(v_sem, 1)
        return fn

    for c in range(NCH):
        if c in pf_pool or c in pb_pe:
            continue
        mk = mk_dve_pB if zone[c] == 'B' else mk_dve_pF
        dve_ops.append((didx[(1, c)], mk(c)))
        n_dve += 1
    for (t, c) in sorted(qb_dve, key=lambda tc: didx[tc]):
        dve_ops.append((didx[(t, c)] + 0.15, mk_dve_qB(t, c)))
        n_dve += 1
    for c in sorted(tail_b):
        for t in (0, 1):
            dve_ops.append((didx[(t, c)] + 0.18, mk_dve_sB(t, c)))
            n_dve += 1
    for tc in sf_units[k['sf_act_n']:]:
        dve_ops.append((didx[tc] + 0.1, mk_dve_sF(*tc)))
        n_dve += 1

    dve_ops.sort(key=lambda x: x[0])
    for _, fn in dve_ops:
        fn()

    # ---------------- PE: zone-B sums + diag squares ----------------
    pe_w = Waits(nc.tensor)
    nc.tensor.wait_ge(c_sem, 3)
    oh = onesh.ap()
    pe_bch = [c for c in pl['bch'] if c not in tail_b]
    pe_units = sorted(((didx[(t, c)], t, c)
                       for c in pe_bch for t in (0, 1)))
    lastS = {t: max((didx[(t, c)], c) for c in pe_bch)[1] for t in (0, 1)}
    qb_pe = [tc for tc in qb_units if tc not in qb_act and tc not in qb_dve]
    lastQ = {}
    for (t, c) in qb_pe:
        if t not in lastQ or didx[(t, c)] > didx[(t, lastQ[t])]:
            lastQ[t] = c
    firstS = {0: True, 1: True}
    firstQ = {0: True, 1: True}
    firstP = [True]
    lastPc = max(pb_pe, key=lambda c: didx[(1, c)]) if pb_pe else None
    n_pe_done = 0
    for _, t, c in pe_units:
        ap, _ = sb(t, c)
        pe_w.need([(t, c)])
        lo = zoff[c]
        sb_full = sbB[t].ap()
        # S matmuls: ones weights, 512-wide windows
        for off in range(lo, lo + CH[c], 512):
            is_last = (c == lastS[t] and off + 512 == lo + CH[c])
            mm = nc.tensor.matmul(
                out=psS[t].ap(), lhsT=oh, rhs=sb_full[:, off:off + 512],
                start=firstS[t], stop=is_last)
            firstS[t] = False
            if is_last:
                mm.then_inc(pe_sem, 1)
                n_pe_done += 1
        # P diag matmuls (on d2 arrival): lhsT = d1 win, rhs = d2 win
        if t == 1 and c in pb_pe:
            pe_w.need([(0, c), (1, c)])
            a0 = sbB[0].ap()
            a1 = sbB[1].ap()
            for off in range(lo, lo + CH[c], 128):
                is_lastp = (c == lastPc and off + 128 == lo + CH[c])
                mm = nc.tensor.matmul(
                    out=psP.ap(), lhsT=a0[:, off:off + 128],
                    rhs=a1[:, off:off + 128], start=firstP[0], stop=is_lastp)
                firstP[0] = False
                if is_lastp:
                    mm.then_inc(pe_sem, 1)
                    n_pe_done += 1
        # Q diag matmuls: 128-wide windows, lhsT = rhs = window
        if (t, c) in qb_act or (t, c) in qb_dve:
            continue
        for off in range(lo, lo + CH[c], 128):
            win = sb_full[:, off:off + 128]
            is_last = (c == lastQ.get(t) and off + 128 == lo + CH[c])
            mm = nc.tensor.matmul(
                out=psQ[t].ap(), lhsT=win, rhs=win,
                start=firstQ[t], stop=is_last)
            firstQ[t] = False
            if is_last:
                mm.then_inc(pe_sem, 1)
                n_pe_done += 1

    # ---------------- end reduces ----------------
    red_on = {'act': ('A', 'A'), 'dve': ('V', 'V'), 'split': ('A', 'V')}[
        k['sred_act'] if isinstance(k['sred_act'], str) else
        ('act' if k['sred_act'] else 'dve')]
    waited = {'A': False, 'V': False}
    for (t, stat), eng in zip(((0, 'S1R'), (1, 'S2R')), red_on):
        if eng == 'A':
            if not waited['A']:
                nc.scalar.wait_ge(pe_sem, n_pe_done)
                waited['A'] = True
            nc.scalar.activation(
                out=scr_a.ap()[:, :512], in_=psS[t].ap(), func=Act.Copy,
                accum_out=col(stat)).then_inc(a_sem, 1)
            n_act += 1
        else:
            if not waited['V']:
                nc.vector.wait_ge(pe_sem, n_pe_done)
                waited['V'] = True
            nc.vector.tensor_scalar(
                out=scr_v.ap()[:, :512], in0=psS[t].ap(), scalar1=0.0,
                scalar2=None, op0=Alu.add, op1=Alu.add,
                accum_out=col(stat)).then_inc(v_sem, 1)
            n_dve += 1
    if pb_pe:
        nc.vector.wait_ge(pe_sem, n_pe_done)
        nc.vector.tensor_scalar(
            out=scr_v.ap()[:, :128], in0=psP.ap(), scalar1=1.0,
            scalar2=None, op0=Alu.mult, op1=Alu.add,
            accum_out=col('scratch'))
        nc.vector.scalar_tensor_tensor(
            out=scr_v.ap()[:, 128:256], in0=scr_v.ap()[:, :128],
            scalar=0.0, in1=ident.ap(), op0=Alu.subtract, op1=Alu.mult,
            accum_out=col('P')).then_inc(v_sem, 1)
        n_dve += 1
    if qb_pe:
        # ttr crashes HW; extract the diagonal via psum-copy + stt*identity
        for t, stat in ((0, 'Q1'), (1, 'Q2')):
            nc.vector.tensor_scalar(
                out=scr_v.ap()[:, :128], in0=psQ[t].ap(), scalar1=1.0,
                scalar2=None, op0=Alu.mult, op1=Alu.add,
                accum_out=col('scratch'))
            nc.vector.scalar_tensor_tensor(
                out=scr_v.ap()[:, 128:256], in0=scr_v.ap()[:, :128],
                scalar=0.0, in1=ident.ap(), op0=Alu.subtract, op1=Alu.mult,
                accum_out=col(stat)).then_inc(v_sem, 1)
            n_dve += 1

    # ---------------- SP: final store; Pool: keepalive ----------------
    nc.sync.wait_ge(v_sem, n_dve)
    if n_act:
        nc.sync.wait_ge(a_sem, n_act)
    if n_pool:
        nc.sync.wait_ge(p_sem, n_pool)
    ov = out[0:P * STATS_COLS].rearrange("(p c) -> p c", p=P)
    nc.sync.dma_start(out=ov, in_=stats.ap()).then_inc(st_sem, 16)

    nc.gpsimd.wait_ge(st_sem, 16)

    nc.finalize()
    return nc, colmap


def _run_device(a1, a2, trace=False, tmpdir=None):
    from concourse.bass_utils import run_bass_kernel_spmd

    sh1 = a1.reshape(NCORES, P, FTOT)
    sh2 = a2.reshape(NCORES, P, FTOT)
    in_maps = [{"d1": sh1[c], "d2": sh2[c]} for c in range(NCORES)]
    global _cached
    if _cached is None:
        _cached = _build()
    nc, colmap = _cached
    res = run_bass_kernel_spmd(
        nc, in_maps, list(range(NCORES)), trace=trace, tmpdir=tmpdir)
    raw = np.stack([res.results[c]["stats"] for c in range(NCORES)])
    return raw, colmap, res


def _combine(raw, colmap):
    t = raw.astype(np.float64).reshape(NCORES, P, STATS_COLS)
    Pc = t[:, :, colmap['P']].sum()
    Q1 = t[:, :, colmap['Q1']].sum()
    Q2 = t[:, :, colmap['Q2']].sum()
    S1 = t[:, :, colmap['S1']].sum() + t[:, 0, colmap['S1R']].sum()
    S2 = t[:, :, colmap['S2']].sum() + t[:, 0, colmap['S2R']].sum()
    n = float(N)
    a1 = S1 / n + 0.001
    a2 = S2 / n + 0.001
    var1 = (Q1 - S1 * S1 / n) / (n - 1)
    var2 = (Q2 - S2 * S2 / n) / (n - 1)
    std1 = np.sqrt(var1)
    std2 = np.sqrt(var2)
    cov = (Pc - a2 * S1 - a1 * S2 + n * a1 * a2) / (n - 1)
    cor = cov / (std1 * std2 + 0.001)
    loss = 0.5 * (cor + 0.001) ** 2
    return np.array([loss], dtype=np.float32)


def kernel(distribution1, distribution2):
    a1 = np.ascontiguousarray(np.asarray(distribution1, dtype=np.float32))
    a2 = np.ascontiguousarray(np.asarray(distribution2, dtype=np.float32))
    raw, colmap, _ = _run_device(a1, a2)
    return _combine(raw, colmap)


# revision 10
# speedup vs baseline: 1.1414x; 1.0422x over previous
"""Trainium2 Bass kernel for nn_CONTRASTLoss v3: zoned fp16/f32 5-engine plan.

CoreSim v1 cost-model facts:
  - DMA occupies its trigger engine for out_free_bytes x 0.3855 ns. Pool
    (SWDGE) DMAs may cast f32->fp16 in flight, halving the charge
    (0.771 ns/col vs 1.542). SP and Act load f32 via HWDGE.
  - DVE fp16 gets 2x/4x modes: tensor_tensor 0.52 ns/col, tensor_scalar
    0.26 ns/col (f32 tensor_scalar: 0.52; two-input f32: 1.0417).
  - PE fp16 matmuls run 1 cycle/row at any p-state (0.4166 ns/col warm);
    LdWeights is free in-model. fp32 data cannot feed fp32r matmuls without
    an explicit rounding pass (BIR verifier), so PE consumes only fp16.
  - Act activation: 0.8533 ns/col (+187 ns accum, +1283 one-time Square
    table load); Pool compute 0.8333 ns/col.

Plan: chunks are zoned 'B' (fp16, cast-loaded by Pool) or 'F' (f32, loaded
by SP/Act). PE computes zone-B sums (ones-weights matmuls -> [128,512] PSUM)
and zone-B squares (diag-trick: lhsT=rhs=window -> [128,128] PSUM, diagonal
extracted once at the end with an affine_select identity mask). DVE computes
all cross-products P (fp16 tt+ts in zone B, f32 stt in zone F) and part of
the f32-zone sums; Act does f32-zone squares + rest of f32 sums; fp16
precision only ever touches zone-B moment partials (~0.02% effect on cov,
tolerance is 2e-2). Host combines raw moments in float64.
"""
import sys

if '/opt/trn_rl_repo' not in sys.path:
    sys.path.insert(0, '/opt/trn_rl_repo')

import numpy as np

N = 16777216
NCORES = 8
P = 128
FTOT = N // NCORES // P          # 16384
CH = [512, 1024, 1536, 1536, 1536, 1536, 1536, 1536, 1536, 1536, 1024, 1024, 512]
assert sum(CH) == FTOT and all(c % 512 == 0 for c in CH)
NCH = len(CH)
CHOFF = np.cumsum([0] + CH).tolist()

KNOBS = dict(
    # zone per chunk: 'B' = fp16 via Pool cast-DMA, 'F' = f32 via SP/Act
    zone='BFBBFBBFBBFBB',
    # S_F units (t,c) assigned to Act (leading count, rest on DVE)
    sf_act_n=5,
    # Q_B units diverted from PE diag to Act squares (leading count)
    qb_act_n=0,
    # P_F chunks diverted from DVE to Pool stt (trailing count)
    pf_pool_n=0,
    # number of B chunks whose P goes to PE (diag on psP) instead of DVE
    pb_pe_n=0,
    # Q_B units diverted from PE diag to DVE tt+ts (trailing count)
    qb_dve_n=0,
    # last n B chunks: S and Q done on DVE (fp16) instead of PE, so the
    # PSUM banks close early and PE leaves the tail critical path
    tail_dve_nch=0,
    # number of F-zone loads given to Act instead of SP (trailing count)
    af_loads_n=0,
    lookahead=4.0,
    act_group=2560, dve_group=4096,
    af_first=True,    # Act loads d2 of the first F chunk (parallel fill)
    sred_act='split',  # 'act' | 'dve' | 'split': psS bank reduce placement
)

STATS_COLS = 48
_cached = None


def _plan(k):
    zone = k['zone']
    assert len(zone) == NCH and set(zone) <= {'B', 'F'}
    bch = [c for c in range(NCH) if zone[c] == 'B']
    fch = [c for c in range(NCH) if zone[c] == 'F']
    # zone-local column offsets
    zoff = {}
    ob = of = 0
    for c in range(NCH):
        if zone[c] == 'B':
            zoff[c] = ob
            ob += CH[c]
        else:
            zoff[c] = of
            of += CH[c]
    btot, ftot = ob, of

    deliv = [(t, c) for c in range(NCH) for t in (0, 1)]
    didx = {tc: i for i, tc in enumerate(deliv)}

    # loads: Pool gets every B (t,c); SP gets F (t,c) except trailing
    # af_loads_n which go to Act
    floads = [(t, c) for (t, c) in deliv if zone[c] == 'F']
    aset = set(floads[len(floads) - k['af_loads_n']:]) if k['af_loads_n'] \
        else set()
    if k.get('af_first') and fch:
        aset.add((1, fch[0]))
    qloads = {'S': [], 'A': [], 'P': []}
    avail = {}
    for (t, c) in deliv:
        q = 'P' if zone[c] == 'B' else ('A' if (t, c) in aset else 'S')
        qloads[q].append((t, c))
        avail[(t, c)] = (q, len(qloads[q]))
    return dict(zone=zone, bch=bch, fch=fch, zoff=zoff, btot=btot, ftot=ftot,
                deliv=deliv, didx=didx, qloads=qloads, avail=avail)


def _build(knobs=None):
    import concourse.bacc as bacc
    import concourse.mybir as mybir

    k = dict(KNOBS)
    if knobs:
        k.update(knobs)

    f32 = mybir.dt.float32
    f16 = mybir.dt.float16
    Alu = mybir.AluOpType
    Act = mybir.ActivationFunctionType
    nc = bacc.Bacc('TRN2', target_bir_lowering=False, debug=False)

    pl = _plan(k)
    zone, zoff = pl['zone'], pl['zoff']
    didx, avail, qloads = pl['didx'], pl['avail'], pl['qloads']

    d1 = nc.declare_dram_parameter("d1", [P, FTOT], f32, isOutput=False)
    d2 = nc.declare_dram_parameter("d2", [P, FTOT], f32, isOutput=False)
    out = nc.declare_dram_parameter("stats", [P * STATS_COLS], f32,
                                    isOutput=True)

    sbF = [nc.alloc_sbuf_tensor(f"sbF{t}", [P, max(pl['ftot'], 512)], f32)
           for t in (0, 1)]
    sbB = [nc.alloc_sbuf_tensor(f"sbB{t}", [P, max(pl['btot'], 512)], f16)
           for t in (0, 1)]
    scr_v = nc.alloc_sbuf_tensor("scr_v", [P, k['dve_group']], f32)
    scr_a = nc.alloc_sbuf_tensor("scr_a", [P, k['act_group']], f32)
    prod = nc.alloc_sbuf_tensor("prod", [P, 2048], f16)
    prod2 = nc.alloc_sbuf_tensor("prod2", [P, 2048], f16)
    stats = nc.alloc_sbuf_tensor("stats_sb", [P, STATS_COLS], f32)
    ones = nc.alloc_sbuf_tensor("ones_sb", [P, 128], f32)
    onesh = nc.alloc_sbuf_tensor("onesh_sb", [P, 128], f16)
    ident = nc.alloc_sbuf_tensor("ident_sb", [P, 128], f32)
    zero = nc.alloc_sbuf_tensor("zero_sb", [P, 1], f32)
    psS = [nc.alloc_psum_tensor(f"psS{t}", [P, 128], f32) for t in (0, 1)]
    psQ = [nc.alloc_psum_tensor(f"psQ{t}", [P, 128], f32) for t in (0, 1)]
    psP = nc.alloc_psum_tensor("psP", [P, 128], f32)
    psD = nc.alloc_psum_tensor("psD", [P, 128], f32)
    nc.const_aps.aps[(f32, 0.0)] = zero.ap()

    qsem = {q: nc.alloc_semaphore(f"q{q}sem") for q in 'SAP'}
    c_sem = nc.alloc_semaphore("c_sem")
    pe_sem = nc.alloc_semaphore("pe_sem")
    v_sem = nc.alloc_semaphore("v_sem")
    a_sem = nc.alloc_semaphore("a_sem")
    p_sem = nc.alloc_semaphore("p_sem")
    st_sem = nc.alloc_semaphore("st_sem")

    drt = [d1, d2]

    colmap = {'P': [], 'Q1': [], 'Q2': [], 'S1': [], 'S2': [],
              'S1R': [], 'S2R': [], 'scratch': []}
    _next_col = [0]

    def col(stat):
        c = _next_col[0]
        _next_col[0] += 1
        assert c < STATS_COLS
        colmap[stat].append(c)
        return stats.ap()[:, c:c + 1]

    class Waits:
        def __init__(self, eng):
            self.eng = eng
            self.seen = {'S': 0, 'A': 0, 'P': 0}

        def need(self, reqs):
            for tc in reqs:
                q, kk = avail[tc]
                if kk > self.seen[q]:
                    self.eng.wait_ge(qsem[q], 16 * kk)
                    self.seen[q] = kk

    def sb(t, c):
        """(tensor, chunk) -> (sbuf AP slice, is_fp16)"""
        lo = zoff[c]
        hi = lo + CH[c]
        if zone[c] == 'B':
            return sbB[t].ap()[:, lo:hi], True
        return sbF[t].ap()[:, lo:hi], False

    def emit_sorted(engine_loads, compute_ops, emit_load):
        items = [(didx[tc] - k['lookahead'], ('L', tc)) for tc in engine_loads]
        items += [(kk + 0.25, ('C', fn)) for (kk, fn) in compute_ops]
        items.sort(key=lambda x: x[0])
        for _, (kind, payload) in items:
            if kind == 'L':
                emit_load(payload)
            else:
                payload()

    def mk_load(eng, q):
        def fn(tc):
            t, c = tc
            lo, hi = CHOFF[c], CHOFF[c + 1]
            ap, _ = sb(t, c)
            eng.dma_start(out=ap, in_=drt[t][:, lo:hi]).then_inc(qsem[q], 16)
        return fn

    # ---------------- consts on gpsimd ----------------
    # onesh (PE weights) must precede the first matmul, so it goes before
    # Pool's first cast load; the remaining consts (Act square bias, extract
    # identity) are only needed much later and are emitted after the first
    # two loads to shave the fill.
    nc.gpsimd.memset(onesh.ap(), 1.0).then_inc(c_sem, 1)
    _late_consts = [False]

    def emit_late_consts():
        if _late_consts[0]:
            return
        _late_consts[0] = True
        nc.gpsimd.memset(ones.ap(), 1.0).then_inc(c_sem, 1)
        nc.gpsimd.memset(zero.ap(), 0.0).then_inc(c_sem, 1)
        nc.gpsimd.affine_select(
            out=ident.ap(), in_=ones.ap(), pattern=[[1, 128]],
            compare_op=Alu.is_equal, fill=0.0, base=0,
            channel_multiplier=-1).then_inc(c_sem, 1)

    # ---------------- SP: f32 loads ----------------
    for tc in qloads['S']:
        mk_load(nc.sync, 'S')(tc)

    # ---------------- work assignment lists ----------------
    # S_F units: (t, c) for F chunks; leading sf_act_n to Act, rest DVE
    sf_units = [(t, c) for c in pl['fch'] for t in (0, 1)]
    sf_units.sort(key=lambda tc: didx[tc])
    sf_act = set(sf_units[:k['sf_act_n']])
    # Q_B units: leading qb_act_n to Act squares, trailing qb_dve_n to DVE,
    # rest PE diag
    qb_units = [(t, c) for c in pl['bch'] for t in (0, 1)]
    qb_units.sort(key=lambda tc: didx[tc])
    qb_act = set(qb_units[:k['qb_act_n']])
    qb_dve = set(qb_units[len(qb_units) - k['qb_dve_n']:]) \
        if k['qb_dve_n'] else set()
    qb_dve -= qb_act
    # P_B chunks on PE: leading pb_pe_n of the B chunk list
    pb_pe = set(pl['bch'][:k['pb_pe_n']])
    # tail B chunks handled fully by DVE
    tail_b = set(pl['bch'][len(pl['bch']) - k['tail_dve_nch']:]) \
        if k['tail_dve_nch'] else set()
    qb_dve |= {(t, c) for c in tail_b for t in (0, 1)}
    qb_dve -= qb_act
    # P_F chunks: trailing pf_pool_n to Pool
    pf_pool = set(pl['fch'][len(pl['fch']) - k['pf_pool_n']:]) \
        if k['pf_pool_n'] else set()

    # ---------------- Act: loads + F squares + S_F copies + QB spill --------
    act_w = Waits(nc.scalar)
    n_act = 0
    act_ops = []

    def fruns(units, cap):
        """Group (t,c) units into runs of consecutive F chunks, same tensor,
        total width <= cap. Units must be F-zone."""
        out_runs = []
        cur = []
        curw = 0
        for (t, c) in units:
            ok = (cur and cur[-1][0] == t and curw + CH[c] <= cap and
                  pl['fch'].index(c) == pl['fch'].index(cur[-1][1]) + 1)
            if ok:
                cur.append((t, c))
                curw += CH[c]
            else:
                if cur:
                    out_runs.append(cur)
                cur = [(t, c)]
                curw = CH[c]
        if cur:
            out_runs.append(cur)
        return out_runs

    def fslice(t, run):
        lo = zoff[run[0][1]]
        hi = zoff[run[-1][1]] + CH[run[-1][1]]
        return sbF[t].ap()[:, lo:hi], hi - lo

    def mk_act_sq(run):
        def fn():
            t = run[0][0]
            ap, w = fslice(t, run)
            act_w.need(run)
            nc.scalar.wait_ge(c_sem, 3)
            nc.scalar.activation(
                out=scr_a.ap()[:, :w], in_=ap, func=Act.Square,
                bias=0.0, scale=1.0,
                accum_out=col('Q1' if t == 0 else 'Q2')).then_inc(a_sem, 1)
        return fn

    def mk_act_sqB(t, c):
        def fn():
            ap, _ = sb(t, c)
            act_w.need([(t, c)])
            nc.scalar.wait_ge(c_sem, 3)
            nc.scalar.activation(
                out=scr_a.ap()[:, :CH[c]], in_=ap, func=Act.Square,
                bias=0.0, scale=1.0,
                accum_out=col('Q1' if t == 0 else 'Q2')).then_inc(a_sem, 1)
        return fn

    def mk_act_scopy(run):
        def fn():
            t = run[0][0]
            ap, w = fslice(t, run)
            act_w.need(run)
            nc.scalar.activation(
                out=scr_a.ap()[:, :w], in_=ap, func=Act.Copy,
                accum_out=col('S1' if t == 0 else 'S2')).then_inc(a_sem, 1)
        return fn

    qf_units = [(t, c) for t in (0, 1) for c in pl['fch']]
    for run in fruns(qf_units, k['act_group']):
        act_ops.append((didx[run[-1]], mk_act_sq(run)))
        n_act += 1
    for run in fruns(sorted(sf_act, key=lambda x: didx[x]), k['act_group']):
        act_ops.append((didx[run[-1]] + 0.1, mk_act_scopy(run)))
        n_act += 1
    for tc in sorted(qb_act, key=lambda x: didx[x]):
        act_ops.append((didx[tc] + 0.05, mk_act_sqB(*tc)))
        n_act += 1
    emit_sorted(qloads['A'], act_ops, mk_load(nc.scalar, 'A'))

    # ---------------- Pool: cast loads + P_F spill + keepalive --------------
    pool_w = Waits(nc.gpsimd)
    n_pool = 0
    pool_ops = []

    def mk_pool_p(c):
        def fn():
            a0, _ = sb(0, c)
            a1, _ = sb(1, c)
            pool_w.need([(0, c), (1, c)])
            nc.gpsimd.scalar_tensor_tensor(
                out=scr_v.ap()[:, :CH[c]], in0=a0, scalar=0.0, in1=a1,
                op0=Alu.subtract, op1=Alu.mult,
                accum_out=col('P')).then_inc(p_sem, 1)
        return fn

    for c in sorted(pf_pool):
        pool_ops.append((didx[(1, c)], mk_pool_p(c)))
        n_pool += 1
    _pl_count = [0]
    _pool_load = mk_load(nc.gpsimd, 'P')

    def pool_load_with_consts(tc):
        _pool_load(tc)
        _pl_count[0] += 1
        if _pl_count[0] == 2:
            emit_late_consts()
    emit_sorted(qloads['P'], pool_ops, pool_load_with_consts)
    emit_late_consts()

    # ---------------- DVE: P everywhere + S_F rest + end reduces ------------
    dve_w = Waits(nc.vector)
    n_dve = 0
    dve_ops = []

    def mk_dve_pB(c):
        def fn():
            a0, _ = sb(0, c)
            a1, _ = sb(1, c)
            dve_w.need([(0, c), (1, c)])
            w = CH[c]
            nc.vector.tensor_tensor(
                out=prod.ap()[:, :w], in0=a0, in1=a1, op=Alu.mult)
            nc.vector.tensor_scalar(
                out=prod2.ap()[:, :w], in0=prod.ap()[:, :w],
                scalar1=1.0, scalar2=None, op0=Alu.mult, op1=Alu.add,
                accum_out=col('P')).then_inc(v_sem, 1)
        return fn

    def mk_dve_pF(c):
        def fn():
            a0, _ = sb(0, c)
            a1, _ = sb(1, c)
            dve_w.need([(0, c), (1, c)])
            nc.vector.scalar_tensor_tensor(
                out=scr_v.ap()[:, :CH[c]], in0=a0, scalar=0.0, in1=a1,
                op0=Alu.subtract, op1=Alu.mult,
                accum_out=col('P')).then_inc(v_sem, 1)
        return fn

    def mk_dve_sF(t, c):
        def fn():
            ap, _ = sb(t, c)
            dve_w.need([(t, c)])
            nc.vector.tensor_scalar(
                out=scr_v.ap()[:, :CH[c]], in0=ap, scalar1=0.0, scalar2=None,
                op0=Alu.add, op1=Alu.add,
                accum_out=col('S1' if t == 0 else 'S2')).then_inc(v_sem, 1)
        return fn

    def mk_dve_sB(t, c):
        def fn():
            ap, _ = sb(t, c)
            dve_w.need([(t, c)])
            w = CH[c]
            nc.vector.tensor_scalar(
                out=prod2.ap()[:, :w], in0=ap, scalar1=1.0, scalar2=None,
                op0=Alu.mult, op1=Alu.add,
                accum_out=col('S1' if t == 0 else 'S2')).then_inc(v_sem, 1)
        return fn

    def mk_dve_qB(t, c):
        def fn():
            ap, _ = sb(t, c)
            dve_w.need([(t, c)])
            w = CH[c]
            nc.vector.tensor_tensor(
                out=prod.ap()[:, :w], in0=ap, in1=ap, op=Alu.mult)
            nc.vector.tensor_scalar(
                out=prod2.ap()[:, :w], in0=prod.ap()[:, :w],
                scalar1=1.0, scalar2=None, op0=Alu.mult, op1=Alu.add,
                accum_out=col('Q1' if t == 0 else 'Q2')).then_inc(v_sem, 1)
        return fn

    for c in range(NCH):
        if c in pf_pool or c in pb_pe:
            continue
        mk = mk_dve_pB if zone[c] == 'B' else mk_dve_pF
        dve_ops.append((didx[(1, c)], mk(c)))
        n_dve += 1
    for (t, c) in sorted(qb_dve, key=lambda tc: didx[tc]):
        dve_ops.append((didx[(t, c)] + 0.15, mk_dve_qB(t, c)))
        n_dve += 1
    for c in sorted(tail_b):
        for t in (0, 1):
            dve_ops.append((didx[(t, c)] + 0.18, mk_dve_sB(t, c)))
            n_dve += 1
    for tc in sf_units[k['sf_act_n']:]:
        dve_ops.append((didx[tc] + 0.1, mk_dve_sF(*tc)))
        n_dve += 1

    dve_ops.sort(key=lambda x: x[0])
    for _, fn in dve_ops:
        fn()

    # ---------------- PE: zone-B sums + diag squares ----------------
    pe_w = Waits(nc.tensor)
    nc.tensor.wait_ge(c_sem, 1)
    # p-state warm-up: keep PE busy through the DMA fill window so the ramp
    # reaches full clock before real data arrives (results unused)
    for _ in range(k.get('pe_warm_n', 0)):
        nc.tensor.matmul(out=psD.ap(), lhsT=onesh.ap(), rhs=onesh.ap(),
                         start=True, stop=True)
    oh = onesh.ap()
    pe_bch = [c for c in pl['bch'] if c not in tail_b]
    pe_units = sorted(((didx[(t, c)], t, c)
                       for c in pe_bch for t in (0, 1)))
    lastS = {t: max((didx[(t, c)], c) for c in pe_bch)[1] for t in (0, 1)}
    qb_pe = [tc for tc in qb_units if tc not in qb_act and tc not in qb_dve]
    lastQ = {}
    for (t, c) in qb_pe:
        if t not in lastQ or didx[(t, c)] > didx[(t, lastQ[t])]:
            lastQ[t] = c
    firstS = {0: True, 1: True}
    firstQ = {0: True, 1: True}
    firstP = [True]
    lastPc = max(pb_pe, key=lambda c: didx[(1, c)]) if pb_pe else None
    n_pe_done = 0
    pe_batch = k.get('pe_batch', 1)
    for ui, (_, t, c) in enumerate(pe_units):
        ap, _ = sb(t, c)
        if pe_batch > 1 and ui % pe_batch == 0:
            batch = [(tt, cc) for (_, tt, cc) in
                     pe_units[ui:ui + pe_batch]]
            pe_w.need(batch)
        else:
            pe_w.need([(t, c)])
        lo = zoff[c]
        sb_full = sbB[t].ap()
        # S matmuls: ones weights, 128-wide windows (cheap end reduce)
        for off in range(lo, lo + CH[c], 128):
            is_last = (c == lastS[t] and off + 128 == lo + CH[c])
            mm = nc.tensor.matmul(
                out=psS[t].ap(), lhsT=oh, rhs=sb_full[:, off:off + 128],
                start=firstS[t], stop=is_last)
            firstS[t] = False
            if is_last:
                mm.then_inc(pe_sem, 1)
                n_pe_done += 1
        # P diag matmuls (on d2 arrival): lhsT = d1 win, rhs = d2 win
        if t == 1 and c in pb_pe:
            pe_w.need([(0, c), (1, c)])
            a0 = sbB[0].ap()
            a1 = sbB[1].ap()
            for off in range(lo, lo + CH[c], 128):
                is_lastp = (c == lastPc and off + 128 == lo + CH[c])
                mm = nc.tensor.matmul(
                    out=psP.ap(), lhsT=a0[:, off:off + 128],
                    rhs=a1[:, off:off + 128], start=firstP[0], stop=is_lastp)
                firstP[0] = False
                if is_lastp:
                    mm.then_inc(pe_sem, 1)
                    n_pe_done += 1
        # Q diag matmuls: 128-wide windows, lhsT = rhs = window
        if (t, c) in qb_act or (t, c) in qb_dve:
            continue
        for off in range(lo, lo + CH[c], 128):
            win = sb_full[:, off:off + 128]
            is_last = (c == lastQ.get(t) and off + 128 == lo + CH[c])
            mm = nc.tensor.matmul(
                out=psQ[t].ap(), lhsT=win, rhs=win,
                start=firstQ[t], stop=is_last)
            firstQ[t] = False
            if is_last:
                mm.then_inc(pe_sem, 1)
                n_pe_done += 1

    # ---------------- end reduces ----------------
    red_on = {'act': ('A', 'A'), 'dve': ('V', 'V'), 'split': ('A', 'V')}[
        k['sred_act'] if isinstance(k['sred_act'], str) else
        ('act' if k['sred_act'] else 'dve')]
    waited = {'A': False, 'V': False}
    for (t, stat), eng in zip(((0, 'S1R'), (1, 'S2R')), red_on):
        if eng == 'A':
            if not waited['A']:
                nc.scalar.wait_ge(pe_sem, n_pe_done)
                waited['A'] = True
            nc.scalar.activation(
                out=scr_a.ap()[:, :128], in_=psS[t].ap(), func=Act.Copy,
                accum_out=col(stat)).then_inc(a_sem, 1)
            n_act += 1
        else:
            if not waited['V']:
                nc.vector.wait_ge(pe_sem, n_pe_done)
                waited['V'] = True
            nc.vector.tensor_scalar(
                out=scr_v.ap()[:, :128], in0=psS[t].ap(), scalar1=0.0,
                scalar2=None, op0=Alu.add, op1=Alu.add,
                accum_out=col(stat)).then_inc(v_sem, 1)
            n_dve += 1
    if pb_pe:
        nc.vector.wait_ge(pe_sem, n_pe_done)
        nc.vector.tensor_scalar(
            out=scr_v.ap()[:, :128], in0=psP.ap(), scalar1=1.0,
            scalar2=None, op0=Alu.mult, op1=Alu.add,
            accum_out=col('scratch'))
        nc.vector.scalar_tensor_tensor(
            out=scr_v.ap()[:, 128:256], in0=scr_v.ap()[:, :128],
            scalar=0.0, in1=ident.ap(), op0=Alu.subtract, op1=Alu.mult,
            accum_out=col('P')).then_inc(v_sem, 1)
        n_dve += 1
    if qb_pe:
        # ttr crashes HW; extract the diagonal via direct stt*identity from
        # PSUM (verified exact on HW)
        nc.vector.wait_ge(c_sem, 4)
        for t, stat in ((0, 'Q1'), (1, 'Q2')):
            nc.vector.scalar_tensor_tensor(
                out=scr_v.ap()[:, :128], in0=psQ[t].ap(),
                scalar=0.0, in1=ident.ap(), op0=Alu.subtract, op1=Alu.mult,
                accum_out=col(stat)).then_inc(v_sem, 1)
            n_dve += 1

    # ---------------- SP: final store; Pool: keepalive ----------------
    nc.sync.wait_ge(v_sem, n_dve)
    if n_act:
        nc.sync.wait_ge(a_sem, n_act)
    if n_pool:
        nc.sync.wait_ge(p_sem, n_pool)
    ov = out[0:P * STATS_COLS].rearrange("(p c) -> p c", p=P)
    nc.sync.dma_start(out=ov, in_=stats.ap()).then_inc(st_sem, 16)

    nc.gpsimd.wait_ge(st_sem, 16)

    nc.finalize()
    return nc, colmap


def _run_device(a1, a2, trace=False, tmpdir=None):
    from concourse.bass_utils import run_bass_kernel_spmd

    sh1 = a1.reshape(NCORES, P, FTOT)
    sh2 = a2.reshape(NCORES, P, FTOT)
    in_maps = [{"d1": sh1[c], "d2": sh2[c]} for c in range(NCORES)]
    global _cached
    if _cached is None:
        _cached = _build()
    nc, colmap = _cached
    res = run_bass_kernel_spmd(
        nc, in_maps, list(range(NCORES)), trace=trace, tmpdir=tmpdir)
    raw = np.stack([res.results[c]["stats"] for c in range(NCORES)])
    return raw, colmap, res


def _combine(raw, colmap):
    t = raw.astype(np.float64).reshape(NCORES, P, STATS_COLS)
    Pc = t[:, :, colmap['P']].sum()
    Q1 = t[:, :, colmap['Q1']].sum()
    Q2 = t[:, :, colmap['Q2']].sum()
    S1 = t[:, :, colmap['S1']].sum() + t[:, 0, colmap['S1R']].sum()
    S2 = t[:, :, colmap['S2']].sum() + t[:, 0, colmap['S2R']].sum()
    n = float(N)
    a1 = S1 / n + 0.001
    a2 = S2 / n + 0.001
    var1 = (Q1 - S1 * S1 / n) / (n - 1)
    var2 = (Q2 - S2 * S2 / n) / (n - 1)
    std1 = np.sqrt(var1)
    std2 = np.sqrt(var2)
    cov = (Pc - a2 * S1 - a1 * S2 + n * a1 * a2) / (n - 1)
    cor = cov / (std1 * std2 + 0.001)
    loss = 0.5 * (cor + 0.001) ** 2
    return np.array([loss], dtype=np.float32)


def kernel(distribution1, distribution2):
    a1 = np.ascontiguousarray(np.asarray(distribution1, dtype=np.float32))
    a2 = np.ascontiguousarray(np.asarray(distribution2, dtype=np.float32))
    raw, colmap, _ = _run_device(a1, a2)
    return _combine(raw, colmap)
